# revision 14
# baseline (speedup 1.0000x reference)
"""Trainium2 Bass kernel for BertLinearSelfAttention (linear attention).

Reference computation (per batch b, head h):
    q,k,v = X @ W{q,k,v} + b{q,k,v}            # [S, D] -> heads of 64
    qf, kf = elu(q)+1, elu(k)+1                # = min(exp(x),1) + max(x,0)
    kv[d,e]  = sum_s kf[s,d] v[s,e]            # [64, 64]
    ksum[d]  = sum_s kf[s,d]
    out[s,e] = (sum_d qf[s,d] kv[d,e]) / (sum_d qf[s,d] ksum[d])

Sharding: 8 cores = (4 batches) x (2 head-groups of 8 heads / 512 proj cols).

Precision: q/k projections run as fp8e4 DoubleRow matmuls (2 contraction
tiles per instruction) with weights pre-scaled by 32; the 1/32 descale is
folded into the feature-map ops.  Their quantization error largely cancels
in the num/den ratio (host-simulated rel_l2 7.7e-3 vs 2e-2 gate).  The v
path stays bf16 (v errors do not cancel).  Optionally (V3) v runs as a
3-term fp8 expansion X8@Wv8 + Xr8@Wv8 + X8@Wvr8 which is as accurate as
bf16 and 25-50% cheaper on the PE depending on the DoubleRow issue rate.

Tail: numerator/denominator matmuls keep the kv-blocks STATIONARY and
stream the q-features (N=512 per instruction), producing a transposed
[cols, tokens] output that the host un-transposes.  This avoids the
per-128-token LDWEIGHTS reloads that made the old tail LDW-bound.  The
denominator uses 32-wide stationaries (2 live cols + 30 zeros) at psum
partition offsets 0/32/64/96 so one [128,512] reciprocal serves all 4
column tiles; per-head recips are partition-broadcast on GPSIMD and
applied to the numerator psum directly on DVE.
"""

import os
import sys

import numpy as np
import ml_dtypes

_REPO = "/opt/trn_rl_repo"
if os.path.isdir(_REPO) and _REPO not in sys.path:
    sys.path.insert(0, _REPO)

B, S, D, H, HD = 4, 4096, 1024, 16, 64
NCORES = 8
CG = 512            # projection columns per core (8 heads)
NH = CG // HD       # 8 heads per core
HE = HD + 2         # head cols incl ksum column + pad
CHUNK = 512         # tokens per chunk
NSUB = CHUNK // 128     # 4 token sub-tiles per chunk
NCHUNK = S // CHUNK     # 8 chunks
NKT = D // 128          # 8 contraction tiles
P = 128
NCT = CG // P           # 4 column tiles (2 heads each)
NKP = NKT // 2          # 4 DoubleRow contraction-tile pairs
WSCALE = 32.0           # fp8 weight pre-scale (power of two)

BF16 = ml_dtypes.bfloat16
E4M3 = ml_dtypes.float8_e4m3

V3 = False              # v projection: False = bf16, True = 3-term fp8

_CACHED_NC = None


def _build():
    import concourse.tile as tile
    from concourse import bacc, mybir
    from contextlib import ExitStack

    F32 = mybir.dt.float32
    BF = mybir.dt.bfloat16
    F8 = mybir.dt.float8e4
    Alu = mybir.AluOpType
    Act = mybir.ActivationFunctionType
    DR = mybir.MatmulPerfMode.DoubleRow
    INV = 1.0 / WSCALE

    nc = bacc.Bacc("TRN2", target_bir_lowering=False, debug=False,
                   num_devices=NCORES)

    # host-packed layouts: rows are SBUF partitions, cols kt-major — every
    # load is one 2D DMA with 128 contiguous multi-KB runs
    x8_d = nc.dram_tensor("x8", [NCHUNK * P, NKT * CHUNK], F8,
                          kind="ExternalInput").ap()
    wk_d = nc.dram_tensor("wk", [P, NKT * CG], F8, kind="ExternalInput").ap()
    wq_d = nc.dram_tensor("wq", [P, NKT * CG], F8, kind="ExternalInput").ap()
    if V3:
        xr_d = nc.dram_tensor("xr", [NCHUNK * P, NKT * CHUNK], F8,
                              kind="ExternalInput").ap()
        wv_d = nc.dram_tensor("wv", [P, NKT * CG], F8,
                              kind="ExternalInput").ap()
        wvr_d = nc.dram_tensor("wvr", [P, NKT * CG], F8,
                               kind="ExternalInput").ap()
    else:
        xb_d = nc.dram_tensor("xb", [NCHUNK * P, NKT * CHUNK], BF,
                              kind="ExternalInput").ap()
        wv_d = nc.dram_tensor("wv", [P, NKT * CG], BF,
                              kind="ExternalInput").ap()
    bq_d = nc.dram_tensor("bq", [CG], F32, kind="ExternalInput").ap()
    bk_d = nc.dram_tensor("bk", [1, CG], BF, kind="ExternalInput").ap()
    bv_d = nc.dram_tensor("bv", [1, NH * HD], F32, kind="ExternalInput").ap()
    # transposed output: [cols, tokens]; host transposes back
    out_d = nc.dram_tensor("out", [CG, S], BF, kind="ExternalOutput").ap()

    with tile.TileContext(nc) as tc:
        with ExitStack() as ctx:
            const = ctx.enter_context(tc.tile_pool(name="const", bufs=1))
            wpool = ctx.enter_context(tc.tile_pool(name="wpool", bufs=1))
            x8pool = ctx.enter_context(tc.tile_pool(name="x8pool", bufs=3))
            if V3:
                xrpool = ctx.enter_context(
                    tc.tile_pool(name="xrpool", bufs=3))
            else:
                xbpool = ctx.enter_context(
                    tc.tile_pool(name="xbpool", bufs=3))

            wk_sb = wpool.tile([P, NKT * CG], F8, tag="wk", name="wk")
            wq_sb = wpool.tile([P, NKT * CG], F8, tag="wq", name="wq")
            if V3:
                wv_sb = wpool.tile([P, NKT * CG], F8, tag="wv", name="wv")
                wvr_sb = wpool.tile([P, NKT * CG], F8, tag="wvr", name="wvr")
            else:
                wv_sb = wpool.tile([P, NKT * CG], BF, tag="wv", name="wv")

            def load_x(ci, pool, dram, dt, eng):
                # split so neither piece is a fully-contiguous DRAM region:
                # whole-slice sources get merged to 1D by the DMA lowering
                # and re-split in an order that scrambles the SBUF dest
                t = pool.tile([P, NKT * CHUNK], dt, tag="x", name="x")
                eng.dma_start(t[:, :CHUNK], dram[ci * P:(ci + 1) * P, :CHUNK])
                eng.dma_start(t[:, CHUNK:], dram[ci * P:(ci + 1) * P, CHUNK:])
                return t

            # startup: smallest pieces first so the first matmul starts
            # ASAP; four queues so descriptor issue parallelizes
            nc.sync.dma_start(wk_sb[:], wk_d[:])
            x8t0 = load_x(0, x8pool, x8_d, F8, nc.sync)
            nc.sync.dma_start(wq_sb[:], wq_d[:])
            if V3:
                nc.gpsimd.dma_start(wv_sb[:], wv_d[:])
                nc.gpsimd.dma_start(wvr_sb[:], wvr_d[:])
                xr0 = load_x(0, xrpool, xr_d, F8, nc.gpsimd)
                x8t1 = load_x(1, x8pool, x8_d, F8, nc.scalar)
                xr1 = load_x(1, xrpool, xr_d, F8, nc.scalar)
            else:
                nc.gpsimd.dma_start(wv_sb[:, :NKT * CG // 2],
                                    wv_d[:, :NKT * CG // 2])
                nc.gpsimd.dma_start(wv_sb[:, NKT * CG // 2:],
                                    wv_d[:, NKT * CG // 2:])
                xb0 = load_x(0, xbpool, xb_d, BF, nc.gpsimd)
                x8t1 = load_x(1, x8pool, x8_d, F8, nc.scalar)
                xb1 = load_x(1, xbpool, xb_d, BF, nc.scalar)

            # ---- small constants ----
            bk_r = const.tile([1, CG], BF, tag="bkr")
            nc.sync.dma_start(bk_r[:], bk_d[:])
            bk_rep = const.tile([P, CG], BF, tag="bkrep")
            nc.gpsimd.partition_broadcast(bk_rep[:], bk_r[:])
            bq_sb = const.tile([P, NCT], F32, tag="bqsb")
            nc.sync.dma_start(bq_sb[:], bq_d.rearrange("(c p) -> p c", p=P))
            bv_sb = const.tile([1, NH * HD], F32, tag="bv32")
            nc.sync.dma_start(bv_sb[:], bv_d[:])
            bv_rep = const.tile([P, NH * HD], F32, tag="bvrep")
            nc.gpsimd.partition_broadcast(bv_rep[:], bv_sb[:])

            # kv accumulator (SBUF f32): per head [64, HE] (ksum in col HD)
            kv_sb = wpool.tile([HD, NH * HE], F32, tag="kvsb")
            nc.vector.memset(kv_sb[:], 0.0)

            # tail stationaries: block-diagonal num weights [128,128] per
            # ct, and den weights [128,128] with ksum REPLICATED across
            # each head's 64 output columns — the den matmul then directly
            # yields a [128,512] psum of per-(col,token) denominators, so
            # no partition broadcast is ever needed (GPSIMD's broadcast
            # only works from partition 0; PE replication is free since
            # matmul cost depends on N, not M)
            kvbn = [wpool.tile([P, P], BF, tag=f"kvbn{i}", name=f"kvbn{i}")
                    for i in range(NCT)]
            kvbd = [wpool.tile([P, P], BF, tag=f"kvbd{i}", name=f"kvbd{i}")
                    for i in range(NCT)]
            for t in kvbn + kvbd:
                nc.vector.memset(t[:], 0.0)

            # persistent V' tiles (2 chunks' worth): tail cols preset once
            vp_tiles = [wpool.tile([P, NH * HE], BF, tag=f"vp{i}",
                                   name=f"vp{i}") for i in range(2 * NSUB)]
            for t in vp_tiles:
                nc.vector.memset(
                    t[:].rearrange("p (h e) -> p h e", e=HE)[:, :, HD:], 0.0)
                nc.vector.memset(
                    t[:].rearrange("p (h e) -> p h e", e=HE)[:, :, HD:HD + 1],
                    1.0)

            # q-feature store for the whole sequence (bf16, 4MB)
            qft_all = wpool.tile([P, NCHUNK * NCT * CHUNK], BF, tag="qft")

            kfpool = ctx.enter_context(tc.tile_pool(name="kfpool", bufs=10))
            tmp = ctx.enter_context(tc.tile_pool(name="tmp", bufs=12))
            outpool = ctx.enter_context(tc.tile_pool(name="outp", bufs=6))
            rcpool = ctx.enter_context(tc.tile_pool(name="rcp", bufs=4))
            pps = ctx.enter_context(
                tc.tile_pool(name="pps", bufs=6, space="PSUM"))
            sps = ctx.enter_context(
                tc.tile_pool(name="sps", bufs=2, space="PSUM"))

            def build_kvblocks():
                # block-diagonal kv (with rank-1 bv fix) + den columns
                for ct in range(NCT):
                    for half in range(2):
                        h = 2 * ct + half
                        dst = kvbn[ct][half * HD:(half + 1) * HD,
                                       half * HD:(half + 1) * HD]
                        ks_col = kv_sb[:, h * HE + HD:h * HE + HD + 1]
                        # kv_fixed = bv_h * ksum_h + kv_h  (rank-1 bias fold)
                        nc.vector.scalar_tensor_tensor(
                            dst, bv_rep[0:HD, h * HD:(h + 1) * HD], ks_col,
                            kv_sb[:, h * HE:h * HE + HD], Alu.mult, Alu.add)
                        nc.vector.tensor_copy(
                            kvbd[ct][half * HD:(half + 1) * HD,
                                     half * HD:(half + 1) * HD],
                            ks_col.broadcast_to([HD, HD]))

            kf_c = {}
            vp_c = {}

            def r3(t):
                return t[:].rearrange("p (k c) -> p k c", k=NKT)

            def do_k(ci, x8t):
                x3 = r3(x8t)
                w3 = r3(wk_sb)
                kfs = []
                for sub in range(NSUB):
                    ps = pps.tile([P, CG], F32, tag="pps", name="kps")
                    for i in range(NKP):
                        nc.tensor.matmul(
                            ps[:],
                            x3[:, 2 * i:2 * i + 2, sub * P:(sub + 1) * P],
                            w3[:, 2 * i:2 * i + 2, :],
                            start=(i == 0), stop=(i == NKP - 1),
                            perf_mode=DR)
                    t = tmp.tile([P, CG], BF, tag="t", name="t_kb")
                    # t = ps/32 + bk
                    nc.vector.scalar_tensor_tensor(
                        t[:], ps[:], INV, bk_rep[:], Alu.mult, Alu.add)
                    e = tmp.tile([P, CG], BF, tag="t", name="t_e")
                    nc.scalar.activation(e[:], t[:], Act.Exp)
                    m = tmp.tile([P, CG], BF, tag="t", name="t_m")
                    nc.vector.tensor_scalar(m[:], e[:], 1.0, None, Alu.min)
                    kf = kfpool.tile([P, CG], BF, tag="kf", name="kf")
                    # kf = max(t,0) + m
                    nc.vector.scalar_tensor_tensor(
                        kf[:], t[:], 0.0, m[:], Alu.max, Alu.add)
                    kfs.append(kf)
                kf_c[ci] = kfs

            def do_v(ci, x8t, xoth):
                vps = []
                if V3:
                    x3, xr3_ = r3(x8t), r3(xoth)
                    wv3, wvr3 = r3(wv_sb), r3(wvr_sb)
                for sub in range(NSUB):
                    ps = pps.tile([P, CG], F32, tag="pps", name="vps")
                    if V3:
                        for i in range(NKP):
                            lhs = x3[:, 2 * i:2 * i + 2,
                                     sub * P:(sub + 1) * P]
                            lhr = xr3_[:, 2 * i:2 * i + 2,
                                       sub * P:(sub + 1) * P]
                            w_ = wv3[:, 2 * i:2 * i + 2, :]
                            wr = wvr3[:, 2 * i:2 * i + 2, :]
                            nc.tensor.matmul(ps[:], lhs, w_,
                                             start=(i == 0), stop=False,
                                             perf_mode=DR)
                            nc.tensor.matmul(ps[:], lhr, w_,
                                             start=False, stop=False,
                                             perf_mode=DR)
                            nc.tensor.matmul(ps[:], lhs, wr,
                                             start=False,
                                             stop=(i == NKP - 1),
                                             perf_mode=DR)
                    else:
                        for kt in range(NKT):
                            c0 = kt * CHUNK + sub * P
                            nc.tensor.matmul(
                                ps[:],
                                xoth[:, c0:c0 + P],
                                wv_sb[:, kt * CG:(kt + 1) * CG],
                                start=(kt == 0), stop=(kt == NKT - 1))
                    vp = vp_tiles[(ci % 2) * NSUB + sub]
                    nc.scalar.activation(
                        vp[:].rearrange("p (h e) -> p h e", e=HE)[:, :, :HD],
                        ps[:].rearrange("p (h e) -> p h e", e=HD),
                        Act.Copy, scale=(INV if V3 else 1.0))
                    vps.append(vp)
                vp_c[ci] = vps

            def do_q(ci, x8t):
                x3 = r3(x8t)
                w3 = r3(wq_sb)
                for ct in range(NCT):
                    ps = pps.tile([P, CHUNK], F32, tag="pps", name="qps")
                    for i in range(NKP):
                        nc.tensor.matmul(
                            ps[:],
                            w3[:, 2 * i:2 * i + 2, ct * P:(ct + 1) * P],
                            x3[:, 2 * i:2 * i + 2, :],
                            start=(i == 0), stop=(i == NKP - 1),
                            perf_mode=DR)
                    bcol = bq_sb[:, ct:ct + 1]
                    e = tmp.tile([P, CHUNK], BF, tag="t", name="t_qe")
                    nc.scalar.activation(e[:], ps[:], Act.Exp, bias=bcol,
                                         scale=INV)
                    m = tmp.tile([P, CHUNK], BF, tag="t", name="t_qm")
                    nc.vector.tensor_scalar(m[:], e[:], 1.0, None, Alu.min)
                    r = tmp.tile([P, CHUNK], BF, tag="t", name="t_qr")
                    # r = ps/32 + bq   (no clamp yet)
                    nc.vector.tensor_scalar(
                        r[:], ps[:], INV, bcol, Alu.mult, Alu.add)
                    q0 = (ci * NCT + ct) * CHUNK
                    # qf = max(r,0) + m
                    nc.vector.scalar_tensor_tensor(
                        qft_all[:, q0:q0 + CHUNK], r[:], 0.0, m[:],
                        Alu.max, Alu.add)

            def do_kv(ci):
                # kv accumulation (head pairs: M=128, N=2*HE)
                kfs, vps = kf_c.pop(ci), vp_c.pop(ci)
                for hp in range(NH // 2):
                    kvt = sps.tile([P, 2 * HE], F32, tag="sps", name="kvt")
                    for sub in range(NSUB):
                        nc.tensor.matmul(
                            kvt[:],
                            kfs[sub][:, hp * P:(hp + 1) * P],
                            vps[sub][:, hp * 2 * HE:(hp + 1) * 2 * HE],
                            start=(sub == 0), stop=(sub == NSUB - 1))
                    # good quadrants: rows 0:64 cols 0:HE (head 2hp),
                    # rows 64:128 cols HE:2HE (head 2hp+1)
                    a0 = (2 * hp) * HE
                    nc.vector.tensor_tensor(
                        kv_sb[:, a0:a0 + HE], kv_sb[:, a0:a0 + HE],
                        kvt[0:HD, 0:HE], Alu.add)
                    a1 = (2 * hp + 1) * HE
                    nc.vector.tensor_tensor(
                        kv_sb[:, a1:a1 + HE], kv_sb[:, a1:a1 + HE],
                        kvt[HD:P, HE:2 * HE], Alu.add)

            # chunk 0/1 interleaved at projection level: k needs only wk,
            # so both chunks' k-projections run while wv/wq still stream in
            if V3:
                xo = {0: xr0, 1: xr1}
            else:
                xo = {0: xb0, 1: xb1}
            x8s = {0: x8t0, 1: x8t1}
            do_k(0, x8s[0])
            do_k(1, x8s[1])
            do_v(0, x8s[0], xo[0])
            do_v(1, x8s[1], xo[1])
            do_q(0, x8s[0])
            do_kv(0)
            do_q(1, x8s[1])
            do_kv(1)
            for ci in range(2, NCHUNK):
                x8t = load_x(ci, x8pool, x8_d, F8, nc.sync)
                if V3:
                    xot = load_x(ci, xrpool, xr_d, F8, nc.gpsimd)
                else:
                    xot = load_x(ci, xbpool, xb_d, BF, nc.gpsimd)
                do_k(ci, x8t)
                do_v(ci, x8t, xot)
                if ci < NCHUNK - 1:
                    do_q(ci, x8t)
                    do_kv(ci)
                else:
                    # last chunk: kv first so the kv-block build (DVE)
                    # overlaps the final q matmuls (PE)
                    do_kv(ci)
                    build_kvblocks()
                    do_q(ci, x8t)

            # ---- tail: per chunk, num/den with stationary kv-blocks ----
            for cj in range(NCHUNK):
                for ct in range(NCT):
                    q0 = (cj * NCT + ct) * CHUNK
                    qsl = qft_all[:, q0:q0 + CHUNK]
                    pdr = pps.tile([P, CHUNK], F32, tag="pps", name="pdr")
                    nc.tensor.matmul(pdr[:], kvbd[ct][:], qsl,
                                     start=True, stop=True)
                    pn = pps.tile([P, CHUNK], F32, tag="pps", name="pn")
                    nc.tensor.matmul(pn[:], kvbn[ct][:], qsl,
                                     start=True, stop=True)
                    rcr = rcpool.tile([P, CHUNK], BF, tag="rc", name="rc")
                    with nc.allow_low_precision(
                            reason="bf16 recip: denominators are O(1e3)"):
                        nc.vector.reciprocal(rcr[:], pdr[:])
                    ot = outpool.tile([P, CHUNK], BF, tag="out", name="osb")
                    nc.vector.tensor_tensor(ot[:], pn[:], rcr[:], Alu.mult)
                    nc.sync.dma_start(
                        out_d[ct * P:(ct + 1) * P,
                              cj * CHUNK:(cj + 1) * CHUNK], ot[:])

    nc.compile()
    return nc


def _get_nc():
    global _CACHED_NC
    if _CACHED_NC is None:
        _CACHED_NC = _build()
    return _CACHED_NC


def _e4(x):
    return np.clip(x, -240.0, 240.0).astype(E4M3)


def _pack_w(w):
    # [D, CG] -> [P, kt-major NKT*CG]: row p, col kt*CG+c = w[kt*P+p, c]
    return np.ascontiguousarray(
        w.reshape(NKT, P, CG).transpose(1, 0, 2).reshape(P, NKT * CG))


def _pack_x(x):
    # [S, D] -> [(chunk, p), kt-major col]:
    # row ci*P+p, col kt*CHUNK+c = X[ci*CHUNK+c, kt*P+p]
    return np.ascontiguousarray(
        x.reshape(NCHUNK, CHUNK, NKT, P)
        .transpose(0, 3, 2, 1).reshape(NCHUNK * P, NKT * CHUNK))


def _make_in_maps(hidden_states, Wq, bq, Wk, bk, Wv, bv):
    hs = np.asarray(hidden_states, np.float32)
    wq = np.asarray(Wq, np.float32)
    wk = np.asarray(Wk, np.float32)
    wv = np.asarray(Wv, np.float32)
    bqf = np.asarray(bq, np.float32)
    bkf = np.asarray(bk, np.float32).astype(BF16)
    bvf = np.asarray(bv, np.float32)
    x8s, xrs, xbs = [], [], []
    for b in range(B):
        xb = hs[b]
        x8 = _e4(xb)
        x8s.append(_pack_x(x8))
        if V3:
            xrs.append(_pack_x(_e4(xb - x8.astype(np.float32))))
        else:
            xbs.append(_pack_x(xb.astype(BF16)))
    in_maps = []
    for c in range(NCORES):
        b, g = divmod(c, 2)
        sl = slice(g * CG, (g + 1) * CG)
        m = {
            "x8": x8s[b],
            "wq": _pack_w(_e4(WSCALE * wq[:, sl])),
            "wk": _pack_w(_e4(WSCALE * wk[:, sl])),
            "bq": np.ascontiguousarray(bqf[sl]),
            "bk": np.ascontiguousarray(bkf[sl]).reshape(1, CG),
            "bv": np.ascontiguousarray(bvf[sl]).reshape(1, CG),
        }
        if V3:
            wvs = WSCALE * wv[:, sl]
            wv8 = _e4(wvs)
            m["xr"] = xrs[b]
            m["wv"] = _pack_w(wv8)
            m["wvr"] = _pack_w(_e4(wvs - wv8.astype(np.float32)))
        else:
            m["xb"] = xbs[b]
            m["wv"] = _pack_w(wv[:, sl].astype(BF16))
        in_maps.append(m)
    return in_maps


def _run(in_maps, **kwargs):
    from concourse.bass_utils import run_bass_kernel_spmd
    nc = _get_nc()
    return run_bass_kernel_spmd(nc, in_maps, core_ids=list(range(NCORES)),
                                **kwargs)


def _assemble(results):
    out = np.empty((B, S, D), np.float32)
    for c in range(NCORES):
        b, g = divmod(c, 2)
        out[b, :, g * CG:(g + 1) * CG] = np.asarray(
            results[c]["out"], dtype=np.float32).T
    return out


def kernel(hidden_states, Wq, bq, Wk, bk, Wv, bv):
    in_maps = _make_in_maps(hidden_states, Wq, bq, Wk, bk, Wv, bv)
    res = _run(in_maps)
    return _assemble(res.results)


# revision 26
# speedup vs baseline: 1.5300x; 1.5300x over previous
"""Trainium2 Bass kernel for BertLinearSelfAttention (linear attention).

Reference computation (per batch b, head h):
    q,k,v = X @ W{q,k,v} + b{q,k,v}            # [S, D] -> heads of 64
    qf, kf = elu(q)+1, elu(k)+1                # = min(exp(x),1) + max(x,0)
    kv[d,e]  = sum_s kf[s,d] v[s,e]            # [64, 64]
    ksum[d]  = sum_s kf[s,d]
    out[s,e] = (sum_d qf[s,d] kv[d,e]) / (sum_d qf[s,d] ksum[d])

Sharding: 8 cores = (4 batches) x (2 head-groups of 8 heads / 512 proj cols).

Precision: q/k projections run as fp8e4 DoubleRow matmuls (2 contraction
tiles per instruction) with weights pre-scaled by 32; the 1/32 descale is
folded into the feature-map ops.  Their quantization error largely cancels
in the num/den ratio (host-simulated rel_l2 7.7e-3 vs 2e-2 gate).  The v
path stays bf16 (v errors do not cancel).  Optionally (V3) v runs as a
3-term fp8 expansion X8@Wv8 + Xr8@Wv8 + X8@Wvr8 which is as accurate as
bf16 and 25-50% cheaper on the PE depending on the DoubleRow issue rate.

Tail: numerator/denominator matmuls keep the kv-blocks STATIONARY and
stream the q-features (N=512 per instruction), producing a transposed
[cols, tokens] output that the host un-transposes.  This avoids the
per-128-token LDWEIGHTS reloads that made the old tail LDW-bound.  The
denominator uses 32-wide stationaries (2 live cols + 30 zeros) at psum
partition offsets 0/32/64/96 so one [128,512] reciprocal serves all 4
column tiles; per-head recips are partition-broadcast on GPSIMD and
applied to the numerator psum directly on DVE.
"""

import os
import sys

import numpy as np
import ml_dtypes

_REPO = "/opt/trn_rl_repo"
if os.path.isdir(_REPO) and _REPO not in sys.path:
    sys.path.insert(0, _REPO)

B, S, D, H, HD = 4, 4096, 1024, 16, 64
NCORES = 8
CG = 512            # projection columns per core (8 heads)
NH = CG // HD       # 8 heads per core
HE = HD + 2         # head cols incl ksum column + pad
CHUNK = 512         # tokens per chunk
NSUB = CHUNK // 128     # 4 token sub-tiles per chunk
NCHUNK = S // CHUNK     # 8 chunks
NKT = D // 128          # 8 contraction tiles
P = 128
NCT = CG // P           # 4 column tiles (2 heads each)
NKP = NKT // 2          # 4 DoubleRow contraction-tile pairs
WSCALE = 32.0           # fp8 weight pre-scale (power of two)

BF16 = ml_dtypes.bfloat16
E4M3 = ml_dtypes.float8_e4m3

V3 = False              # v projection: False = bf16, True = 3-term fp8

_CACHED_NC = None


def _build():
    import concourse.tile as tile
    from concourse import bacc, mybir
    from contextlib import ExitStack

    F32 = mybir.dt.float32
    BF = mybir.dt.bfloat16
    F8 = mybir.dt.float8e4
    Alu = mybir.AluOpType
    Act = mybir.ActivationFunctionType
    DR = mybir.MatmulPerfMode.DoubleRow
    INV = 1.0 / WSCALE

    nc = bacc.Bacc("TRN2", target_bir_lowering=False, debug=False,
                   num_devices=NCORES)

    # host-packed layouts: rows are SBUF partitions, cols kt-major — every
    # load is one 2D DMA with 128 contiguous multi-KB runs
    x8_d = nc.dram_tensor("x8", [NCHUNK * P, NKT * CHUNK], F8,
                          kind="ExternalInput").ap()
    wk_d = nc.dram_tensor("wk", [P, NKT * CG], F8, kind="ExternalInput").ap()
    wq_d = nc.dram_tensor("wq", [P, NKT * CG], F8, kind="ExternalInput").ap()
    if V3:
        xr_d = nc.dram_tensor("xr", [NCHUNK * P, NKT * CHUNK], F8,
                              kind="ExternalInput").ap()
        wv_d = nc.dram_tensor("wv", [P, NKT * CG], F8,
                              kind="ExternalInput").ap()
        wvr_d = nc.dram_tensor("wvr", [P, NKT * CG], F8,
                               kind="ExternalInput").ap()
    else:
        xb_d = nc.dram_tensor("xb", [NCHUNK * P, NKT * CHUNK], BF,
                              kind="ExternalInput").ap()
        wv_d = nc.dram_tensor("wv", [P, NKT * CG], BF,
                              kind="ExternalInput").ap()
    bq_d = nc.dram_tensor("bq", [CG], F32, kind="ExternalInput").ap()
    bk_d = nc.dram_tensor("bk", [1, CG], BF, kind="ExternalInput").ap()
    bv_d = nc.dram_tensor("bv", [1, NH * HD], F32, kind="ExternalInput").ap()
    # transposed output: [cols, tokens]; host transposes back
    out_d = nc.dram_tensor("out", [CG, S], BF, kind="ExternalOutput").ap()

    with tile.TileContext(nc) as tc:
        with ExitStack() as ctx:
            const = ctx.enter_context(tc.tile_pool(name="const", bufs=1))
            wpool = ctx.enter_context(tc.tile_pool(name="wpool", bufs=1))
            x8pool = ctx.enter_context(tc.tile_pool(name="x8pool", bufs=3))
            if V3:
                xrpool = ctx.enter_context(
                    tc.tile_pool(name="xrpool", bufs=3))
            else:
                xbpool = ctx.enter_context(
                    tc.tile_pool(name="xbpool", bufs=3))

            wk_sb = wpool.tile([P, NKT * CG], F8, tag="wk", name="wk")
            wq_sb = wpool.tile([P, NKT * CG], F8, tag="wq", name="wq")
            if V3:
                wv_sb = wpool.tile([P, NKT * CG], F8, tag="wv", name="wv")
                wvr_sb = wpool.tile([P, NKT * CG], F8, tag="wvr", name="wvr")
            else:
                wv_sb = wpool.tile([P, NKT * CG], BF, tag="wv", name="wv")

            def load_x(ci, pool, dram, dt, eng):
                # split so neither piece is a fully-contiguous DRAM region:
                # whole-slice sources get merged to 1D by the DMA lowering
                # and re-split in an order that scrambles the SBUF dest
                t = pool.tile([P, NKT * CHUNK], dt, tag="x", name="x")
                eng.dma_start(t[:, :CHUNK], dram[ci * P:(ci + 1) * P, :CHUNK])
                eng.dma_start(t[:, CHUNK:], dram[ci * P:(ci + 1) * P, CHUNK:])
                return t

            # startup: smallest pieces first so the first matmul starts
            # ASAP; four queues so descriptor issue parallelizes
            nc.sync.dma_start(wk_sb[:], wk_d[:])
            x8t0 = load_x(0, x8pool, x8_d, F8, nc.sync)
            nc.sync.dma_start(wq_sb[:], wq_d[:])
            if V3:
                nc.gpsimd.dma_start(wv_sb[:], wv_d[:])
                nc.gpsimd.dma_start(wvr_sb[:], wvr_d[:])
                xr0 = load_x(0, xrpool, xr_d, F8, nc.gpsimd)
                x8t1 = load_x(1, x8pool, x8_d, F8, nc.scalar)
                xr1 = load_x(1, xrpool, xr_d, F8, nc.scalar)
            else:
                nc.gpsimd.dma_start(wv_sb[:, :NKT * CG // 2],
                                    wv_d[:, :NKT * CG // 2])
                nc.gpsimd.dma_start(wv_sb[:, NKT * CG // 2:],
                                    wv_d[:, NKT * CG // 2:])
                xb0 = load_x(0, xbpool, xb_d, BF, nc.gpsimd)
                x8t1 = load_x(1, x8pool, x8_d, F8, nc.scalar)
                xb1 = load_x(1, xbpool, xb_d, BF, nc.scalar)

            # ---- small constants ----
            # bk arrives pre-scaled by 32 (host); the 1/32 descale is
            # applied by the ACT ops' scale argument
            bk_r = const.tile([1, CG], BF, tag="bkr")
            nc.sync.dma_start(bk_r[:], bk_d[:])
            bk_rep = const.tile([P, CG], BF, tag="bkrep")
            nc.gpsimd.partition_broadcast(bk_rep[:], bk_r[:])
            bq_sb = const.tile([P, NCT], F32, tag="bqsb")
            nc.sync.dma_start(bq_sb[:], bq_d.rearrange("(c p) -> p c", p=P))
            bv_sb = const.tile([1, NH * HD], F32, tag="bv32")
            nc.sync.dma_start(bv_sb[:], bv_d[:])
            bv_rep = const.tile([P, NH * HD], F32, tag="bvrep")
            nc.gpsimd.partition_broadcast(bv_rep[:], bv_sb[:])

            # kv accumulator (SBUF f32): per head [64, HE] (ksum in col HD)
            kv_sb = wpool.tile([HD, NH * HE], F32, tag="kvsb")
            nc.vector.memset(kv_sb[:], 0.0)

            # tail stationaries: block-diagonal num weights [128,128] per
            # ct, and den weights [128,128] with ksum REPLICATED across
            # each head's 64 output columns — the den matmul then directly
            # yields a [128,512] psum of per-(col,token) denominators, so
            # no partition broadcast is ever needed (GPSIMD's broadcast
            # only works from partition 0; PE replication is free since
            # matmul cost depends on N, not M)
            kvbn = [wpool.tile([P, P], BF, tag=f"kvbn{i}", name=f"kvbn{i}")
                    for i in range(NCT)]
            kvbd = [wpool.tile([P, P], BF, tag=f"kvbd{i}", name=f"kvbd{i}")
                    for i in range(NCT)]
            for t in kvbn + kvbd:
                nc.vector.memset(t[:], 0.0)

            # persistent V' tiles (2 chunks' worth): tail cols preset once
            vp_tiles = [wpool.tile([P, NH * HE], BF, tag=f"vp{i}",
                                   name=f"vp{i}") for i in range(2 * NSUB)]
            for t in vp_tiles:
                nc.vector.memset(
                    t[:].rearrange("p (h e) -> p h e", e=HE)[:, :, HD:], 0.0)
                nc.vector.memset(
                    t[:].rearrange("p (h e) -> p h e", e=HE)[:, :, HD:HD + 1],
                    1.0)

            # q-feature store for the whole sequence (bf16, 4MB)
            qft_all = wpool.tile([P, NCHUNK * NCT * CHUNK], BF, tag="qft")

            kfpool = ctx.enter_context(tc.tile_pool(name="kfpool", bufs=10))
            tmp = ctx.enter_context(tc.tile_pool(name="tmp", bufs=12))
            outpool = ctx.enter_context(tc.tile_pool(name="outp", bufs=6))
            rcpool = ctx.enter_context(tc.tile_pool(name="rcp", bufs=4))
            pps = ctx.enter_context(
                tc.tile_pool(name="pps", bufs=6, space="PSUM"))
            sps = ctx.enter_context(
                tc.tile_pool(name="sps", bufs=2, space="PSUM"))

            def build_kvblocks():
                # block-diagonal kv (with rank-1 bv fix) + den columns
                for ct in range(NCT):
                    for half in range(2):
                        h = 2 * ct + half
                        dst = kvbn[ct][half * HD:(half + 1) * HD,
                                       half * HD:(half + 1) * HD]
                        ks_col = kv_sb[:, h * HE + HD:h * HE + HD + 1]
                        # kv_fixed = bv_h * ksum_h + kv_h  (rank-1 bias fold)
                        nc.vector.scalar_tensor_tensor(
                            dst, bv_rep[0:HD, h * HD:(h + 1) * HD], ks_col,
                            kv_sb[:, h * HE:h * HE + HD], Alu.mult, Alu.add)
                        nc.vector.tensor_copy(
                            kvbd[ct][half * HD:(half + 1) * HD,
                                     half * HD:(half + 1) * HD],
                            ks_col.broadcast_to([HD, HD]))

            kf_c = {}
            vp_c = {}

            def r3(t):
                return t[:].rearrange("p (k c) -> p k c", k=NKT)

            def do_k(ci, x8t):
                x3 = r3(x8t)
                w3 = r3(wk_sb)
                kfs = []
                for sub in range(NSUB):
                    ps = pps.tile([P, CG], F32, tag="pps", name="kps")
                    for i in range(NKP):
                        nc.tensor.matmul(
                            ps[:],
                            x3[:, 2 * i:2 * i + 2, sub * P:(sub + 1) * P],
                            w3[:, 2 * i:2 * i + 2, :],
                            start=(i == 0), stop=(i == NKP - 1),
                            perf_mode=DR)
                    # t' = ps + 32*bk (one-ALU-op DVE; descale on ACT)
                    t = tmp.tile([P, CG], BF, tag="t", name="t_kb")
                    nc.vector.tensor_tensor(t[:], ps[:], bk_rep[:], Alu.add)
                    e = tmp.tile([P, CG], BF, tag="t", name="t_e")
                    nc.scalar.activation(e[:], t[:], Act.Exp, scale=INV)
                    r = tmp.tile([P, CG], BF, tag="t", name="t_r")
                    nc.scalar.activation(r[:], t[:], Act.Relu, scale=INV)
                    m = tmp.tile([P, CG], BF, tag="t", name="t_m")
                    nc.vector.tensor_scalar(m[:], e[:], 1.0, None, Alu.min)
                    kf = kfpool.tile([P, CG], BF, tag="kf", name="kf")
                    nc.vector.tensor_tensor(kf[:], m[:], r[:], Alu.add)
                    kfs.append(kf)
                kf_c[ci] = kfs

            def do_v(ci, x8t, xoth):
                vps = []
                if V3:
                    x3, xr3_ = r3(x8t), r3(xoth)
                    wv3, wvr3 = r3(wv_sb), r3(wvr_sb)
                for sub in range(NSUB):
                    ps = pps.tile([P, CG], F32, tag="pps", name="vps")
                    if V3:
                        for i in range(NKP):
                            lhs = x3[:, 2 * i:2 * i + 2,
                                     sub * P:(sub + 1) * P]
                            lhr = xr3_[:, 2 * i:2 * i + 2,
                                       sub * P:(sub + 1) * P]
                            w_ = wv3[:, 2 * i:2 * i + 2, :]
                            wr = wvr3[:, 2 * i:2 * i + 2, :]
                            nc.tensor.matmul(ps[:], lhs, w_,
                                             start=(i == 0), stop=False,
                                             perf_mode=DR)
                            nc.tensor.matmul(ps[:], lhr, w_,
                                             start=False, stop=False,
                                             perf_mode=DR)
                            nc.tensor.matmul(ps[:], lhs, wr,
                                             start=False,
                                             stop=(i == NKP - 1),
                                             perf_mode=DR)
                    else:
                        for kt in range(NKT):
                            c0 = kt * CHUNK + sub * P
                            nc.tensor.matmul(
                                ps[:],
                                xoth[:, c0:c0 + P],
                                wv_sb[:, kt * CG:(kt + 1) * CG],
                                start=(kt == 0), stop=(kt == NKT - 1))
                    vp = vp_tiles[(ci % 2) * NSUB + sub]
                    if V3:
                        nc.scalar.activation(
                            vp[:].rearrange("p (h e) -> p h e",
                                            e=HE)[:, :, :HD],
                            ps[:].rearrange("p (h e) -> p h e", e=HD),
                            Act.Copy, scale=INV)
                    else:
                        # DVE copy: ACT is the tighter engine in the
                        # main loop
                        nc.vector.tensor_copy(
                            vp[:].rearrange("p (h e) -> p h e",
                                            e=HE)[:, :, :HD],
                            ps[:].rearrange("p (h e) -> p h e", e=HD))
                    vps.append(vp)
                vp_c[ci] = vps

            def do_q(ci, x8t):
                x3 = r3(x8t)
                w3 = r3(wq_sb)
                for ct in range(NCT):
                    ps = pps.tile([P, CHUNK], F32, tag="pps", name="qps")
                    for i in range(NKP):
                        nc.tensor.matmul(
                            ps[:],
                            w3[:, 2 * i:2 * i + 2, ct * P:(ct + 1) * P],
                            x3[:, 2 * i:2 * i + 2, :],
                            start=(i == 0), stop=(i == NKP - 1),
                            perf_mode=DR)
                    bcol = bq_sb[:, ct:ct + 1]
                    e = tmp.tile([P, CHUNK], BF, tag="t", name="t_qe")
                    nc.scalar.activation(e[:], ps[:], Act.Exp, bias=bcol,
                                         scale=INV)
                    r = tmp.tile([P, CHUNK], BF, tag="t", name="t_qr")
                    nc.scalar.activation(r[:], ps[:], Act.Relu, bias=bcol,
                                         scale=INV)
                    m = tmp.tile([P, CHUNK], BF, tag="t", name="t_qm")
                    nc.vector.tensor_scalar(m[:], e[:], 1.0, None, Alu.min)
                    q0 = (ci * NCT + ct) * CHUNK
                    nc.vector.tensor_tensor(
                        qft_all[:, q0:q0 + CHUNK], m[:], r[:], Alu.add)

            def do_kv_pair(ci):
                # kv accumulation for chunk pair (ci-1, ci): one psum
                # accumulation group of 8 matmuls per head pair, each in
                # its own bank (regions must NOT share banks: a second
                # region's start=True wipes the first region's data)
                kfs0, vps0 = kf_c.pop(ci - 1), vp_c.pop(ci - 1)
                kfs1, vps1 = kf_c.pop(ci), vp_c.pop(ci)
                for hp in range(NH // 2):
                    kvt = sps.tile([P, 2 * HE], F32, tag="sps", name="kvt")
                    for j, (kfs, vps) in enumerate(((kfs0, vps0),
                                                    (kfs1, vps1))):
                        for sub in range(NSUB):
                            nc.tensor.matmul(
                                kvt[:],
                                kfs[sub][:, hp * P:(hp + 1) * P],
                                vps[sub][:, hp * 2 * HE:(hp + 1) * 2 * HE],
                                start=(j == 0 and sub == 0),
                                stop=(j == 1 and sub == NSUB - 1))
                    # good quadrants: rows 0:64 cols 0:HE (head 2hp),
                    # rows 64:128 cols HE:2HE (head 2hp+1)
                    a0 = (2 * hp) * HE
                    nc.vector.tensor_tensor(
                        kv_sb[:, a0:a0 + HE], kv_sb[:, a0:a0 + HE],
                        kvt[0:HD, 0:HE], Alu.add)
                    a1 = (2 * hp + 1) * HE
                    nc.vector.tensor_tensor(
                        kv_sb[:, a1:a1 + HE], kv_sb[:, a1:a1 + HE],
                        kvt[HD:P, HE:2 * HE], Alu.add)

            # chunk 0/1 interleaved at projection level: k needs only wk,
            # so both chunks' k-projections run while wv/wq still stream in
            if V3:
                xo = {0: xr0, 1: xr1}
            else:
                xo = {0: xb0, 1: xb1}
            x8s = {0: x8t0, 1: x8t1}
            do_k(0, x8s[0])
            do_k(1, x8s[1])
            do_v(0, x8s[0], xo[0])
            do_v(1, x8s[1], xo[1])
            do_q(0, x8s[0])
            do_q(1, x8s[1])
            do_kv_pair(1)
            for ci in range(2, NCHUNK):
                x8t = load_x(ci, x8pool, x8_d, F8, nc.sync)
                if V3:
                    xot = load_x(ci, xrpool, xr_d, F8, nc.gpsimd)
                else:
                    xot = load_x(ci, xbpool, xb_d, BF, nc.gpsimd)
                do_k(ci, x8t)
                do_v(ci, x8t, xot)
                if ci < NCHUNK - 1:
                    do_q(ci, x8t)
                    if ci % 2 == 1:
                        do_kv_pair(ci)
                else:
                    # last chunk: kv first so the kv-block build (DVE)
                    # overlaps the final q matmuls (PE)
                    do_kv_pair(ci)
                    build_kvblocks()
                    do_q(ci, x8t)

            # ---- tail: per chunk, num/den with stationary kv-blocks ----
            for cj in range(NCHUNK):
                for ct in range(NCT):
                    q0 = (cj * NCT + ct) * CHUNK
                    qsl = qft_all[:, q0:q0 + CHUNK]
                    pdr = pps.tile([P, CHUNK], F32, tag="pps", name="pdr")
                    nc.tensor.matmul(pdr[:], kvbd[ct][:], qsl,
                                     start=True, stop=True)
                    pn = pps.tile([P, CHUNK], F32, tag="pps", name="pn")
                    nc.tensor.matmul(pn[:], kvbn[ct][:], qsl,
                                     start=True, stop=True)
                    rcr = rcpool.tile([P, CHUNK], F32, tag="rc", name="rc")
                    # dens are O(1e3) positive sums: no recip edge cases
                    nc.vector.reciprocal_approx_fast(rcr[:], pdr[:])
                    ot = outpool.tile([P, CHUNK], BF, tag="out", name="osb")
                    nc.vector.tensor_tensor(ot[:], pn[:], rcr[:], Alu.mult)
                    nc.sync.dma_start(
                        out_d[ct * P:(ct + 1) * P,
                              cj * CHUNK:(cj + 1) * CHUNK], ot[:])

    nc.compile()
    return nc


def _get_nc():
    global _CACHED_NC
    if _CACHED_NC is None:
        _CACHED_NC = _build()
    return _CACHED_NC


def _e4(x):
    return np.clip(x, -240.0, 240.0).astype(E4M3)


def _pack_w(w):
    # [D, CG] -> [P, kt-major NKT*CG]: row p, col kt*CG+c = w[kt*P+p, c]
    return np.ascontiguousarray(
        w.reshape(NKT, P, CG).transpose(1, 0, 2).reshape(P, NKT * CG))


def _pack_x(x):
    # [S, D] -> [(chunk, p), kt-major col]:
    # row ci*P+p, col kt*CHUNK+c = X[ci*CHUNK+c, kt*P+p]
    return np.ascontiguousarray(
        x.reshape(NCHUNK, CHUNK, NKT, P)
        .transpose(0, 3, 2, 1).reshape(NCHUNK * P, NKT * CHUNK))


def _make_in_maps(hidden_states, Wq, bq, Wk, bk, Wv, bv):
    hs = np.asarray(hidden_states, np.float32)
    wq = np.asarray(Wq, np.float32)
    wk = np.asarray(Wk, np.float32)
    wv = np.asarray(Wv, np.float32)
    bqf = np.asarray(bq, np.float32)
    # bk is added to the psum BEFORE the 1/32 descale -> pre-scale by 32
    bkf = (WSCALE * np.asarray(bk, np.float32)).astype(BF16)
    bvf = np.asarray(bv, np.float32)
    x8s, xrs, xbs = [], [], []
    for b in range(B):
        xb = hs[b]
        x8 = _e4(xb)
        x8s.append(_pack_x(x8))
        if V3:
            xrs.append(_pack_x(_e4(xb - x8.astype(np.float32))))
        else:
            xbs.append(_pack_x(xb.astype(BF16)))
    in_maps = []
    for c in range(NCORES):
        b, g = divmod(c, 2)
        sl = slice(g * CG, (g + 1) * CG)
        m = {
            "x8": x8s[b],
            "wq": _pack_w(_e4(WSCALE * wq[:, sl])),
            "wk": _pack_w(_e4(WSCALE * wk[:, sl])),
            "bq": np.ascontiguousarray(bqf[sl]),
            "bk": np.ascontiguousarray(bkf[sl]).reshape(1, CG),
            "bv": np.ascontiguousarray(bvf[sl]).reshape(1, CG),
        }
        if V3:
            wvs = WSCALE * wv[:, sl]
            wv8 = _e4(wvs)
            m["xr"] = xrs[b]
            m["wv"] = _pack_w(wv8)
            m["wvr"] = _pack_w(_e4(wvs - wv8.astype(np.float32)))
        else:
            m["xb"] = xbs[b]
            m["wv"] = _pack_w(wv[:, sl].astype(BF16))
        in_maps.append(m)
    return in_maps


def _run(in_maps, **kwargs):
    from concourse.bass_utils import run_bass_kernel_spmd
    nc = _get_nc()
    return run_bass_kernel_spmd(nc, in_maps, core_ids=list(range(NCORES)),
                                **kwargs)


def _assemble(results):
    out = np.empty((B, S, D), np.float32)
    for c in range(NCORES):
        b, g = divmod(c, 2)
        out[b, :, g * CG:(g + 1) * CG] = np.asarray(
            results[c]["out"], dtype=np.float32).T
    return out


def kernel(hidden_states, Wq, bq, Wk, bk, Wv, bv):
    in_maps = _make_in_maps(hidden_states, Wq, bq, Wk, bk, Wv, bv)
    res = _run(in_maps)
    return _assemble(res.results)


# revision 31
# speedup vs baseline: 1.7052x; 1.1145x over previous
"""Trainium2 Bass kernel for BertLinearSelfAttention (linear attention).

Reference computation (per batch b, head h):
    q,k,v = X @ W{q,k,v} + b{q,k,v}            # [S, D] -> heads of 64
    qf, kf = elu(q)+1, elu(k)+1                # = min(exp(x),1) + max(x,0)
    kv[d,e]  = sum_s kf[s,d] v[s,e]            # [64, 64]
    ksum[d]  = sum_s kf[s,d]
    out[s,e] = (sum_d qf[s,d] kv[d,e]) / (sum_d qf[s,d] ksum[d])

Sharding: 8 cores = (4 batches) x (2 head-groups of 8 heads / 512 proj cols).

Precision: q/k projections run as fp8e4 DoubleRow matmuls (2 contraction
tiles per instruction) with weights pre-scaled by 32; the 1/32 descale is
folded into the feature-map ops.  Their quantization error largely cancels
in the num/den ratio (host-simulated rel_l2 7.7e-3 vs 2e-2 gate).  The v
path stays bf16 (v errors do not cancel).  Optionally (V3) v runs as a
3-term fp8 expansion X8@Wv8 + Xr8@Wv8 + X8@Wvr8 which is as accurate as
bf16 and 25-50% cheaper on the PE depending on the DoubleRow issue rate.

Tail: numerator/denominator matmuls keep the kv-blocks STATIONARY and
stream the q-features (N=512 per instruction), producing a transposed
[cols, tokens] output that the host un-transposes.  This avoids the
per-128-token LDWEIGHTS reloads that made the old tail LDW-bound.  The
denominator uses 32-wide stationaries (2 live cols + 30 zeros) at psum
partition offsets 0/32/64/96 so one [128,512] reciprocal serves all 4
column tiles; per-head recips are partition-broadcast on GPSIMD and
applied to the numerator psum directly on DVE.
"""

import os
import sys

import numpy as np
import ml_dtypes

_REPO = "/opt/trn_rl_repo"
if os.path.isdir(_REPO) and _REPO not in sys.path:
    sys.path.insert(0, _REPO)

B, S, D, H, HD = 4, 4096, 1024, 16, 64
NCORES = 8
CG = 512            # projection columns per core (8 heads)
NH = CG // HD       # 8 heads per core
HE = HD + 2         # head cols incl ksum column + pad
CHUNK = 512         # tokens per chunk
NSUB = CHUNK // 128     # 4 token sub-tiles per chunk
NCHUNK = S // CHUNK     # 8 chunks
NKT = D // 128          # 8 contraction tiles
P = 128
NCT = CG // P           # 4 column tiles (2 heads each)
NKP = NKT // 2          # 4 DoubleRow contraction-tile pairs
WSCALE = 32.0           # fp8 weight pre-scale (power of two)

BF16 = ml_dtypes.bfloat16
E4M3 = ml_dtypes.float8_e4m3

V3 = False              # v projection: False = bf16, True = 3-term fp8

_CACHED_NC = None


def _build():
    import concourse.tile as tile
    from concourse import bacc, mybir
    from contextlib import ExitStack

    F32 = mybir.dt.float32
    BF = mybir.dt.bfloat16
    F8 = mybir.dt.float8e4
    Alu = mybir.AluOpType
    Act = mybir.ActivationFunctionType
    DR = mybir.MatmulPerfMode.DoubleRow
    INV = 1.0 / WSCALE

    nc = bacc.Bacc("TRN2", target_bir_lowering=False, debug=False,
                   num_devices=NCORES)

    # host-packed layouts: rows are SBUF partitions, cols kt-major — every
    # load is one 2D DMA with 128 contiguous multi-KB runs
    x8_d = nc.dram_tensor("x8", [NCHUNK * P, NKT * CHUNK], F8,
                          kind="ExternalInput").ap()
    wk_d = nc.dram_tensor("wk", [P, NKT * CG], F8, kind="ExternalInput").ap()
    wq_d = nc.dram_tensor("wq", [P, NKT * CG], F8, kind="ExternalInput").ap()
    if V3:
        xr_d = nc.dram_tensor("xr", [NCHUNK * P, NKT * CHUNK], F8,
                              kind="ExternalInput").ap()
        wv_d = nc.dram_tensor("wv", [P, NKT * CG], F8,
                              kind="ExternalInput").ap()
        wvr_d = nc.dram_tensor("wvr", [P, NKT * CG], F8,
                               kind="ExternalInput").ap()
    else:
        xb_d = nc.dram_tensor("xb", [NCHUNK * P, NKT * CHUNK], BF,
                              kind="ExternalInput").ap()
        wv_d = nc.dram_tensor("wv", [P, NKT * CG], BF,
                              kind="ExternalInput").ap()
    bq_d = nc.dram_tensor("bq", [CG], F32, kind="ExternalInput").ap()
    bk_d = nc.dram_tensor("bk", [1, CG], BF, kind="ExternalInput").ap()
    bv_d = nc.dram_tensor("bv", [1, NH * HD], F32, kind="ExternalInput").ap()
    out_d = nc.dram_tensor("out", [S, CG], BF, kind="ExternalOutput").ap()

    with tile.TileContext(nc) as tc:
        with ExitStack() as ctx:
            const = ctx.enter_context(tc.tile_pool(name="const", bufs=1))
            wpool = ctx.enter_context(tc.tile_pool(name="wpool", bufs=1))
            x8pool = ctx.enter_context(tc.tile_pool(name="x8pool", bufs=3))
            if V3:
                xrpool = ctx.enter_context(
                    tc.tile_pool(name="xrpool", bufs=3))
            else:
                xbpool = ctx.enter_context(
                    tc.tile_pool(name="xbpool", bufs=3))

            wk_sb = wpool.tile([P, NKT * CG], F8, tag="wk", name="wk")
            wq_sb = wpool.tile([P, NKT * CG], F8, tag="wq", name="wq")
            if V3:
                wv_sb = wpool.tile([P, NKT * CG], F8, tag="wv", name="wv")
                wvr_sb = wpool.tile([P, NKT * CG], F8, tag="wvr", name="wvr")
            else:
                wv_sb = wpool.tile([P, NKT * CG], BF, tag="wv", name="wv")

            def load_x(ci, pool, dram, dt, eng):
                # split so neither piece is a fully-contiguous DRAM region:
                # whole-slice sources get merged to 1D by the DMA lowering
                # and re-split in an order that scrambles the SBUF dest
                t = pool.tile([P, NKT * CHUNK], dt, tag="x", name="x")
                eng.dma_start(t[:, :CHUNK], dram[ci * P:(ci + 1) * P, :CHUNK])
                eng.dma_start(t[:, CHUNK:], dram[ci * P:(ci + 1) * P, CHUNK:])
                return t

            # startup: smallest pieces first so the first matmul starts
            # ASAP; four queues so descriptor issue parallelizes
            nc.sync.dma_start(wk_sb[:], wk_d[:])
            x8t0 = load_x(0, x8pool, x8_d, F8, nc.sync)
            nc.sync.dma_start(wq_sb[:], wq_d[:])
            if V3:
                nc.gpsimd.dma_start(wv_sb[:], wv_d[:])
                nc.gpsimd.dma_start(wvr_sb[:], wvr_d[:])
                xr0 = load_x(0, xrpool, xr_d, F8, nc.gpsimd)
                x8t1 = load_x(1, x8pool, x8_d, F8, nc.scalar)
                xr1 = load_x(1, xrpool, xr_d, F8, nc.scalar)
            else:
                nc.gpsimd.dma_start(wv_sb[:, :NKT * CG // 2],
                                    wv_d[:, :NKT * CG // 2])
                nc.gpsimd.dma_start(wv_sb[:, NKT * CG // 2:],
                                    wv_d[:, NKT * CG // 2:])
                xb0 = load_x(0, xbpool, xb_d, BF, nc.gpsimd)
                x8t1 = load_x(1, x8pool, x8_d, F8, nc.scalar)
                xb1 = load_x(1, xbpool, xb_d, BF, nc.scalar)

            # ---- small constants ----
            # bk arrives pre-scaled by 32 (host); the 1/32 descale is
            # applied by the ACT ops' scale argument
            bk_r = const.tile([1, CG], BF, tag="bkr")
            nc.sync.dma_start(bk_r[:], bk_d[:])
            bk_rep = const.tile([P, CG], BF, tag="bkrep")
            nc.gpsimd.partition_broadcast(bk_rep[:], bk_r[:])
            bq_sb = const.tile([P, NCT], F32, tag="bqsb")
            nc.sync.dma_start(bq_sb[:], bq_d.rearrange("(c p) -> p c", p=P))
            bv_sb = const.tile([1, NH * HD], F32, tag="bv32")
            nc.sync.dma_start(bv_sb[:], bv_d[:])
            bv_rep = const.tile([P, NH * HD], F32, tag="bvrep")
            nc.gpsimd.partition_broadcast(bv_rep[:], bv_sb[:])

            # kv accumulator (SBUF f32): per head [64, HE] (ksum in col HD)
            kv_sb = wpool.tile([HD, NH * HE], F32, tag="kvsb")
            nc.vector.memset(kv_sb[:], 0.0)

            # tail stationaries: per ct a [128, 130] moving block — cols
            # 0:128 block-diagonal kv (2 heads), cols 128/129 the two
            # heads' ksum columns.  One matmul per (sub, ct) with the
            # qft slice stationary yields numerator AND denominator in
            # token-major orientation, sharing one LDWEIGHTS.
            kvbn = [wpool.tile([P, 130], BF, tag=f"kvbn{i}", name=f"kvbn{i}")
                    for i in range(NCT)]
            for t in kvbn:
                nc.vector.memset(t[:], 0.0)

            # persistent V' tiles (2 chunks' worth): tail cols preset once
            vp_tiles = [wpool.tile([P, NH * HE], BF, tag=f"vp{i}",
                                   name=f"vp{i}") for i in range(2 * NSUB)]
            for t in vp_tiles:
                nc.vector.memset(
                    t[:].rearrange("p (h e) -> p h e", e=HE)[:, :, HD:], 0.0)
                nc.vector.memset(
                    t[:].rearrange("p (h e) -> p h e", e=HE)[:, :, HD:HD + 1],
                    1.0)

            # q-feature store for the whole sequence (bf16, 4MB)
            qft_all = wpool.tile([P, NCHUNK * NCT * CHUNK], BF, tag="qft")

            kfpool = ctx.enter_context(tc.tile_pool(name="kfpool", bufs=10))
            tmp = ctx.enter_context(tc.tile_pool(name="tmp", bufs=12))
            outpool = ctx.enter_context(tc.tile_pool(name="outp", bufs=6))
            rcpool = ctx.enter_context(tc.tile_pool(name="rcp", bufs=4))
            pps = ctx.enter_context(
                tc.tile_pool(name="pps", bufs=6, space="PSUM"))
            sps = ctx.enter_context(
                tc.tile_pool(name="sps", bufs=2, space="PSUM"))

            def build_kvblocks():
                # block-diagonal kv (with rank-1 bv fix) + den columns
                for ct in range(NCT):
                    for half in range(2):
                        h = 2 * ct + half
                        dst = kvbn[ct][half * HD:(half + 1) * HD,
                                       half * HD:(half + 1) * HD]
                        ks_col = kv_sb[:, h * HE + HD:h * HE + HD + 1]
                        # kv_fixed = bv_h * ksum_h + kv_h  (rank-1 bias fold)
                        nc.vector.scalar_tensor_tensor(
                            dst, bv_rep[0:HD, h * HD:(h + 1) * HD], ks_col,
                            kv_sb[:, h * HE:h * HE + HD], Alu.mult, Alu.add)
                        nc.vector.tensor_copy(
                            kvbn[ct][half * HD:(half + 1) * HD,
                                     128 + half:129 + half],
                            ks_col)

            kf_c = {}
            vp_c = {}

            def r3(t):
                return t[:].rearrange("p (k c) -> p k c", k=NKT)

            def do_k(ci, x8t):
                x3 = r3(x8t)
                w3 = r3(wk_sb)
                kfs = []
                for sub in range(NSUB):
                    ps = pps.tile([P, CG], F32, tag="pps", name="kps")
                    for i in range(NKP):
                        nc.tensor.matmul(
                            ps[:],
                            x3[:, 2 * i:2 * i + 2, sub * P:(sub + 1) * P],
                            w3[:, 2 * i:2 * i + 2, :],
                            start=(i == 0), stop=(i == NKP - 1),
                            perf_mode=DR)
                    # t' = ps + 32*bk (one-ALU-op DVE; descale on ACT)
                    t = tmp.tile([P, CG], BF, tag="t", name="t_kb")
                    nc.vector.tensor_tensor(t[:], ps[:], bk_rep[:], Alu.add)
                    e = tmp.tile([P, CG], BF, tag="t", name="t_e")
                    nc.scalar.activation(e[:], t[:], Act.Exp, scale=INV)
                    r = tmp.tile([P, CG], BF, tag="t", name="t_r")
                    nc.scalar.activation(r[:], t[:], Act.Relu, scale=INV)
                    m = tmp.tile([P, CG], BF, tag="t", name="t_m")
                    nc.vector.tensor_scalar(m[:], e[:], 1.0, None, Alu.min)
                    kf = kfpool.tile([P, CG], BF, tag="kf", name="kf")
                    nc.vector.tensor_tensor(kf[:], m[:], r[:], Alu.add)
                    kfs.append(kf)
                kf_c[ci] = kfs

            def do_v(ci, x8t, xoth):
                vps = []
                if V3:
                    x3, xr3_ = r3(x8t), r3(xoth)
                    wv3, wvr3 = r3(wv_sb), r3(wvr_sb)
                for sub in range(NSUB):
                    ps = pps.tile([P, CG], F32, tag="pps", name="vps")
                    if V3:
                        for i in range(NKP):
                            lhs = x3[:, 2 * i:2 * i + 2,
                                     sub * P:(sub + 1) * P]
                            lhr = xr3_[:, 2 * i:2 * i + 2,
                                       sub * P:(sub + 1) * P]
                            w_ = wv3[:, 2 * i:2 * i + 2, :]
                            wr = wvr3[:, 2 * i:2 * i + 2, :]
                            nc.tensor.matmul(ps[:], lhs, w_,
                                             start=(i == 0), stop=False,
                                             perf_mode=DR)
                            nc.tensor.matmul(ps[:], lhr, w_,
                                             start=False, stop=False,
                                             perf_mode=DR)
                            nc.tensor.matmul(ps[:], lhs, wr,
                                             start=False,
                                             stop=(i == NKP - 1),
                                             perf_mode=DR)
                    else:
                        for kt in range(NKT):
                            c0 = kt * CHUNK + sub * P
                            nc.tensor.matmul(
                                ps[:],
                                xoth[:, c0:c0 + P],
                                wv_sb[:, kt * CG:(kt + 1) * CG],
                                start=(kt == 0), stop=(kt == NKT - 1))
                    vp = vp_tiles[(ci % 2) * NSUB + sub]
                    if V3:
                        nc.scalar.activation(
                            vp[:].rearrange("p (h e) -> p h e",
                                            e=HE)[:, :, :HD],
                            ps[:].rearrange("p (h e) -> p h e", e=HD),
                            Act.Copy, scale=INV)
                    else:
                        # DVE copy: ACT is the tighter engine in the
                        # main loop
                        nc.vector.tensor_copy(
                            vp[:].rearrange("p (h e) -> p h e",
                                            e=HE)[:, :, :HD],
                            ps[:].rearrange("p (h e) -> p h e", e=HD))
                    vps.append(vp)
                vp_c[ci] = vps

            def do_q(ci, x8t):
                x3 = r3(x8t)
                w3 = r3(wq_sb)
                for ct in range(NCT):
                    ps = pps.tile([P, CHUNK], F32, tag="pps", name="qps")
                    for i in range(NKP):
                        nc.tensor.matmul(
                            ps[:],
                            w3[:, 2 * i:2 * i + 2, ct * P:(ct + 1) * P],
                            x3[:, 2 * i:2 * i + 2, :],
                            start=(i == 0), stop=(i == NKP - 1),
                            perf_mode=DR)
                    bcol = bq_sb[:, ct:ct + 1]
                    e = tmp.tile([P, CHUNK], BF, tag="t", name="t_qe")
                    nc.scalar.activation(e[:], ps[:], Act.Exp, bias=bcol,
                                         scale=INV)
                    r = tmp.tile([P, CHUNK], BF, tag="t", name="t_qr")
                    nc.scalar.activation(r[:], ps[:], Act.Relu, bias=bcol,
                                         scale=INV)
                    m = tmp.tile([P, CHUNK], BF, tag="t", name="t_qm")
                    nc.vector.tensor_scalar(m[:], e[:], 1.0, None, Alu.min)
                    q0 = (ci * NCT + ct) * CHUNK
                    nc.vector.tensor_tensor(
                        qft_all[:, q0:q0 + CHUNK], m[:], r[:], Alu.add)

            def do_kv_pair(ci):
                # kv accumulation for chunk pair (ci-1, ci): one psum
                # accumulation group of 8 matmuls per head pair, each in
                # its own bank (regions must NOT share banks: a second
                # region's start=True wipes the first region's data)
                kfs0, vps0 = kf_c.pop(ci - 1), vp_c.pop(ci - 1)
                kfs1, vps1 = kf_c.pop(ci), vp_c.pop(ci)
                for hp in range(NH // 2):
                    kvt = sps.tile([P, 2 * HE], F32, tag="sps", name="kvt")
                    for j, (kfs, vps) in enumerate(((kfs0, vps0),
                                                    (kfs1, vps1))):
                        for sub in range(NSUB):
                            nc.tensor.matmul(
                                kvt[:],
                                kfs[sub][:, hp * P:(hp + 1) * P],
                                vps[sub][:, hp * 2 * HE:(hp + 1) * 2 * HE],
                                start=(j == 0 and sub == 0),
                                stop=(j == 1 and sub == NSUB - 1))
                    # good quadrants: rows 0:64 cols 0:HE (head 2hp),
                    # rows 64:128 cols HE:2HE (head 2hp+1)
                    a0 = (2 * hp) * HE
                    nc.vector.tensor_tensor(
                        kv_sb[:, a0:a0 + HE], kv_sb[:, a0:a0 + HE],
                        kvt[0:HD, 0:HE], Alu.add)
                    a1 = (2 * hp + 1) * HE
                    nc.vector.tensor_tensor(
                        kv_sb[:, a1:a1 + HE], kv_sb[:, a1:a1 + HE],
                        kvt[HD:P, HE:2 * HE], Alu.add)

            # chunk 0/1 interleaved at projection level: k needs only wk,
            # so both chunks' k-projections run while wv/wq still stream in
            if V3:
                xo = {0: xr0, 1: xr1}
            else:
                xo = {0: xb0, 1: xb1}
            x8s = {0: x8t0, 1: x8t1}
            do_k(0, x8s[0])
            do_k(1, x8s[1])
            do_v(0, x8s[0], xo[0])
            do_v(1, x8s[1], xo[1])
            do_q(0, x8s[0])
            do_q(1, x8s[1])
            do_kv_pair(1)
            for ci in range(2, NCHUNK):
                x8t = load_x(ci, x8pool, x8_d, F8, nc.sync)
                if V3:
                    xot = load_x(ci, xrpool, xr_d, F8, nc.gpsimd)
                else:
                    xot = load_x(ci, xbpool, xb_d, BF, nc.gpsimd)
                do_k(ci, x8t)
                do_v(ci, x8t, xot)
                if ci < NCHUNK - 1:
                    do_q(ci, x8t)
                    if ci % 2 == 1:
                        do_kv_pair(ci)
                else:
                    # last chunk: kv first so the kv-block build (DVE)
                    # overlaps the final q matmuls (PE)
                    do_kv_pair(ci)
                    build_kvblocks()
                    do_q(ci, x8t)

            # ---- tail: token-major fused num+den, N=130 per matmul ----
            for cj in range(NCHUNK):
                tok0 = cj * CHUNK
                for sub in range(NSUB):
                    for g in range(2):
                        # T packs ct=2g (cols 0:130) and ct=2g+1 (130:260)
                        T = pps.tile([P, 260], F32, tag="pps", name="tps")
                        for half in range(2):
                            ct = 2 * g + half
                            q0 = (cj * NCT + ct) * CHUNK + sub * P
                            nc.tensor.matmul(
                                T[:, half * 130:(half + 1) * 130],
                                qft_all[:, q0:q0 + P], kvbn[ct][:],
                                start=True, stop=True)
                        T3 = T[:].rearrange("p (a c) -> p a c", c=130)
                        rc4 = rcpool.tile([P, 4], BF, tag="rc", name="rc")
                        with nc.allow_low_precision(
                                reason="bf16 recip: dens are O(1e3) sums"):
                            nc.vector.reciprocal(
                                rc4[:].rearrange("p (a c) -> p a c", c=2),
                                T3[:, :, 128:130])
                        # psum f32 -> sbuf bf16 on ACT; mult all-bf16 on DVE
                        pnc = tmp.tile([P, 256], BF, tag="t", name="pnc")
                        nc.scalar.copy(
                            pnc[:].rearrange("p (a c) -> p a c", c=128),
                            T3[:, :, 0:128])
                        ot = outpool.tile([P, 256], BF, tag="out",
                                          name="osb")
                        rcb = rc4[:].unsqueeze(2).broadcast_to([P, 4, HD])
                        nc.vector.tensor_tensor(
                            ot[:].rearrange("p (h e) -> p h e", e=HD),
                            pnc[:].rearrange("p (h e) -> p h e", e=HD),
                            rcb, Alu.mult)
                        nc.sync.dma_start(
                            out_d[tok0 + sub * P:tok0 + (sub + 1) * P,
                                  g * 256:(g + 1) * 256], ot[:])

    nc.compile()
    return nc


def _get_nc():
    global _CACHED_NC
    if _CACHED_NC is None:
        _CACHED_NC = _build()
    return _CACHED_NC


def _e4(x):
    return np.clip(x, -240.0, 240.0).astype(E4M3)


def _pack_w(w):
    # [D, CG] -> [P, kt-major NKT*CG]: row p, col kt*CG+c = w[kt*P+p, c]
    return np.ascontiguousarray(
        w.reshape(NKT, P, CG).transpose(1, 0, 2).reshape(P, NKT * CG))


def _pack_x(x):
    # [S, D] -> [(chunk, p), kt-major col]:
    # row ci*P+p, col kt*CHUNK+c = X[ci*CHUNK+c, kt*P+p]
    return np.ascontiguousarray(
        x.reshape(NCHUNK, CHUNK, NKT, P)
        .transpose(0, 3, 2, 1).reshape(NCHUNK * P, NKT * CHUNK))


def _make_in_maps(hidden_states, Wq, bq, Wk, bk, Wv, bv):
    hs = np.asarray(hidden_states, np.float32)
    wq = np.asarray(Wq, np.float32)
    wk = np.asarray(Wk, np.float32)
    wv = np.asarray(Wv, np.float32)
    bqf = np.asarray(bq, np.float32)
    # bk is added to the psum BEFORE the 1/32 descale -> pre-scale by 32
    bkf = (WSCALE * np.asarray(bk, np.float32)).astype(BF16)
    bvf = np.asarray(bv, np.float32)
    x8s, xrs, xbs = [], [], []
    for b in range(B):
        xb = hs[b]
        x8 = _e4(xb)
        x8s.append(_pack_x(x8))
        if V3:
            xrs.append(_pack_x(_e4(xb - x8.astype(np.float32))))
        else:
            xbs.append(_pack_x(xb.astype(BF16)))
    in_maps = []
    for c in range(NCORES):
        b, g = divmod(c, 2)
        sl = slice(g * CG, (g + 1) * CG)
        m = {
            "x8": x8s[b],
            "wq": _pack_w(_e4(WSCALE * wq[:, sl])),
            "wk": _pack_w(_e4(WSCALE * wk[:, sl])),
            "bq": np.ascontiguousarray(bqf[sl]),
            "bk": np.ascontiguousarray(bkf[sl]).reshape(1, CG),
            "bv": np.ascontiguousarray(bvf[sl]).reshape(1, CG),
        }
        if V3:
            wvs = WSCALE * wv[:, sl]
            wv8 = _e4(wvs)
            m["xr"] = xrs[b]
            m["wv"] = _pack_w(wv8)
            m["wvr"] = _pack_w(_e4(wvs - wv8.astype(np.float32)))
        else:
            m["xb"] = xbs[b]
            m["wv"] = _pack_w(wv[:, sl].astype(BF16))
        in_maps.append(m)
    return in_maps


def _run(in_maps, **kwargs):
    from concourse.bass_utils import run_bass_kernel_spmd
    nc = _get_nc()
    return run_bass_kernel_spmd(nc, in_maps, core_ids=list(range(NCORES)),
                                **kwargs)


def _assemble(results):
    out = np.empty((B, S, D), np.float32)
    for c in range(NCORES):
        b, g = divmod(c, 2)
        out[b, :, g * CG:(g + 1) * CG] = np.asarray(
            results[c]["out"], dtype=np.float32)
    return out


def kernel(hidden_states, Wq, bq, Wk, bk, Wv, bv):
    in_maps = _make_in_maps(hidden_states, Wq, bq, Wk, bk, Wv, bv)
    res = _run(in_maps)
    return _assemble(res.results)


# revision 33
# speedup vs baseline: 1.9164x; 1.1239x over previous
"""Trainium2 Bass kernel for BertLinearSelfAttention (linear attention).

Reference computation (per batch b, head h):
    q,k,v = X @ W{q,k,v} + b{q,k,v}            # [S, D] -> heads of 64
    qf, kf = elu(q)+1, elu(k)+1                # = min(exp(x),1) + max(x,0)
    kv[d,e]  = sum_s kf[s,d] v[s,e]            # [64, 64]
    ksum[d]  = sum_s kf[s,d]
    out[s,e] = (sum_d qf[s,d] kv[d,e]) / (sum_d qf[s,d] ksum[d])

Sharding: 8 cores = (4 batches) x (2 head-groups of 8 heads / 512 proj cols).

Precision: q/k projections run as fp8e4 DoubleRow matmuls (2 contraction
tiles per instruction) with weights pre-scaled by 32; the 1/32 descale is
folded into the feature-map ops.  Their quantization error largely cancels
in the num/den ratio (host-simulated rel_l2 7.7e-3 vs 2e-2 gate).  The v
path stays bf16 (v errors do not cancel).  Optionally (V3) v runs as a
3-term fp8 expansion X8@Wv8 + Xr8@Wv8 + X8@Wvr8 which is as accurate as
bf16 and 25-50% cheaper on the PE depending on the DoubleRow issue rate.

Tail: numerator/denominator matmuls keep the kv-blocks STATIONARY and
stream the q-features (N=512 per instruction), producing a transposed
[cols, tokens] output that the host un-transposes.  This avoids the
per-128-token LDWEIGHTS reloads that made the old tail LDW-bound.  The
denominator uses 32-wide stationaries (2 live cols + 30 zeros) at psum
partition offsets 0/32/64/96 so one [128,512] reciprocal serves all 4
column tiles; per-head recips are partition-broadcast on GPSIMD and
applied to the numerator psum directly on DVE.
"""

import os
import sys

import numpy as np
import ml_dtypes

_REPO = "/opt/trn_rl_repo"
if os.path.isdir(_REPO) and _REPO not in sys.path:
    sys.path.insert(0, _REPO)

B, S, D, H, HD = 4, 4096, 1024, 16, 64
NCORES = 8
CG = 512            # projection columns per core (8 heads)
NH = CG // HD       # 8 heads per core
HE = HD + 2         # head cols incl ksum column + pad
CHUNK = 512         # tokens per chunk
NSUB = CHUNK // 128     # 4 token sub-tiles per chunk
NCHUNK = S // CHUNK     # 8 chunks
NKT = D // 128          # 8 contraction tiles
P = 128
NCT = CG // P           # 4 column tiles (2 heads each)
NKP = NKT // 2          # 4 DoubleRow contraction-tile pairs
WSCALE = 32.0           # fp8 weight pre-scale (power of two)

BF16 = ml_dtypes.bfloat16
E4M3 = ml_dtypes.float8_e4m3

V3 = False              # v projection: False = bf16, True = 3-term fp8

_CACHED_NC = None


def _build():
    import concourse.tile as tile
    from concourse import bacc, mybir
    from contextlib import ExitStack

    F32 = mybir.dt.float32
    BF = mybir.dt.bfloat16
    F8 = mybir.dt.float8e4
    Alu = mybir.AluOpType
    Act = mybir.ActivationFunctionType
    DR = mybir.MatmulPerfMode.DoubleRow
    INV = 1.0 / WSCALE

    nc = bacc.Bacc("TRN2", target_bir_lowering=False, debug=False,
                   num_devices=NCORES)

    # host-packed layouts: rows are SBUF partitions, cols kt-major — every
    # load is one 2D DMA with 128 contiguous multi-KB runs
    x8_d = nc.dram_tensor("x8", [NCHUNK * P, NKT * CHUNK], F8,
                          kind="ExternalInput").ap()
    wk_d = nc.dram_tensor("wk", [P, NKT * CG], F8, kind="ExternalInput").ap()
    wq_d = nc.dram_tensor("wq", [P, NKT * CG], F8, kind="ExternalInput").ap()
    if V3:
        xr_d = nc.dram_tensor("xr", [NCHUNK * P, NKT * CHUNK], F8,
                              kind="ExternalInput").ap()
        wv_d = nc.dram_tensor("wv", [P, NKT * CG], F8,
                              kind="ExternalInput").ap()
        wvr_d = nc.dram_tensor("wvr", [P, NKT * CG], F8,
                               kind="ExternalInput").ap()
    else:
        xb_d = nc.dram_tensor("xb", [NCHUNK * P, NKT * CHUNK], BF,
                              kind="ExternalInput").ap()
        wv_d = nc.dram_tensor("wv", [P, NKT * CG], BF,
                              kind="ExternalInput").ap()
    bq_d = nc.dram_tensor("bq", [CG], F32, kind="ExternalInput").ap()
    bk_d = nc.dram_tensor("bk", [1, CG], BF, kind="ExternalInput").ap()
    bv_d = nc.dram_tensor("bv", [1, NH * HD], F32, kind="ExternalInput").ap()
    out_d = nc.dram_tensor("out", [S, CG], BF, kind="ExternalOutput").ap()

    with tile.TileContext(nc) as tc:
        with ExitStack() as ctx:
            const = ctx.enter_context(tc.tile_pool(name="const", bufs=1))
            wpool = ctx.enter_context(tc.tile_pool(name="wpool", bufs=1))
            x8pool = ctx.enter_context(tc.tile_pool(name="x8pool", bufs=3))
            if V3:
                xrpool = ctx.enter_context(
                    tc.tile_pool(name="xrpool", bufs=3))
            else:
                xbpool = ctx.enter_context(
                    tc.tile_pool(name="xbpool", bufs=3))

            wk_sb = wpool.tile([P, NKT * CG], F8, tag="wk", name="wk")
            wq_sb = wpool.tile([P, NKT * CG], F8, tag="wq", name="wq")
            if V3:
                wv_sb = wpool.tile([P, NKT * CG], F8, tag="wv", name="wv")
                wvr_sb = wpool.tile([P, NKT * CG], F8, tag="wvr", name="wvr")
            else:
                wv_sb = wpool.tile([P, NKT * CG], BF, tag="wv", name="wv")

            def load_x(ci, pool, dram, dt, eng, pieces=2):
                # split so neither piece is a fully-contiguous DRAM region:
                # whole-slice sources get merged to 1D by the DMA lowering
                # and re-split in an order that scrambles the SBUF dest
                t = pool.tile([P, NKT * CHUNK], dt, tag="x", name="x")
                W = NKT * CHUNK // pieces
                for j in range(pieces):
                    eng.dma_start(t[:, j * W:(j + 1) * W],
                                  dram[ci * P:(ci + 1) * P, j * W:(j + 1) * W])
                return t

            # startup: kt-pair-granular pieces so the first DoubleRow
            # matmuls start as soon as their contraction slices land
            nc.sync.dma_start(wk_sb[:, :NKT * CG // 4], wk_d[:, :NKT * CG // 4])
            nc.sync.dma_start(wk_sb[:, NKT * CG // 4:], wk_d[:, NKT * CG // 4:])
            x8t0 = load_x(0, x8pool, x8_d, F8, nc.sync, pieces=4)
            nc.sync.dma_start(wq_sb[:], wq_d[:])
            if V3:
                nc.gpsimd.dma_start(wv_sb[:], wv_d[:])
                nc.gpsimd.dma_start(wvr_sb[:], wvr_d[:])
                xr0 = load_x(0, xrpool, xr_d, F8, nc.gpsimd)
                x8t1 = load_x(1, x8pool, x8_d, F8, nc.scalar)
                xr1 = load_x(1, xrpool, xr_d, F8, nc.scalar)
            else:
                nc.gpsimd.dma_start(wv_sb[:, :NKT * CG // 2],
                                    wv_d[:, :NKT * CG // 2])
                nc.gpsimd.dma_start(wv_sb[:, NKT * CG // 2:],
                                    wv_d[:, NKT * CG // 2:])
                xb0 = load_x(0, xbpool, xb_d, BF, nc.gpsimd)
                x8t1 = load_x(1, x8pool, x8_d, F8, nc.scalar)
                xb1 = load_x(1, xbpool, xb_d, BF, nc.scalar)

            # ---- small constants ----
            # bk arrives pre-scaled by 32 (host); the 1/32 descale is
            # applied by the ACT ops' scale argument
            bk_r = const.tile([1, CG], BF, tag="bkr")
            nc.sync.dma_start(bk_r[:], bk_d[:])
            bk_rep = const.tile([P, CG], BF, tag="bkrep")
            nc.gpsimd.partition_broadcast(bk_rep[:], bk_r[:])
            bq_sb = const.tile([P, NCT], F32, tag="bqsb")
            nc.sync.dma_start(bq_sb[:], bq_d.rearrange("(c p) -> p c", p=P))
            bv_sb = const.tile([1, NH * HD], F32, tag="bv32")
            nc.sync.dma_start(bv_sb[:], bv_d[:])
            bv_rep = const.tile([P, NH * HD], F32, tag="bvrep")
            nc.gpsimd.partition_broadcast(bv_rep[:], bv_sb[:])

            # kv accumulator (SBUF f32): per head [64, HE] (ksum in col HD)
            kv_sb = wpool.tile([HD, NH * HE], F32, tag="kvsb")
            nc.vector.memset(kv_sb[:], 0.0)

            # tail stationaries: per ct a [128, 130] moving block — cols
            # 0:128 block-diagonal kv (2 heads), cols 128/129 the two
            # heads' ksum columns.  One matmul per (sub, ct) with the
            # qft slice stationary yields numerator AND denominator in
            # token-major orientation, sharing one LDWEIGHTS.
            kvbn = [wpool.tile([P, 130], BF, tag=f"kvbn{i}", name=f"kvbn{i}")
                    for i in range(NCT)]
            for t in kvbn:
                nc.vector.memset(t[:], 0.0)

            # persistent V' tiles (2 chunks' worth): tail cols preset once
            vp_tiles = [wpool.tile([P, NH * HE], BF, tag=f"vp{i}",
                                   name=f"vp{i}") for i in range(2 * NSUB)]
            for t in vp_tiles:
                nc.vector.memset(
                    t[:].rearrange("p (h e) -> p h e", e=HE)[:, :, HD:], 0.0)
                nc.vector.memset(
                    t[:].rearrange("p (h e) -> p h e", e=HE)[:, :, HD:HD + 1],
                    1.0)

            # q-feature store for the whole sequence (bf16, 4MB)
            qft_all = wpool.tile([P, NCHUNK * NCT * CHUNK], BF, tag="qft")

            kfpool = ctx.enter_context(tc.tile_pool(name="kfpool", bufs=10))
            tmp = ctx.enter_context(tc.tile_pool(name="tmp", bufs=12))
            outpool = ctx.enter_context(tc.tile_pool(name="outp", bufs=6))
            rcpool = ctx.enter_context(tc.tile_pool(name="rcp", bufs=4))
            pps = ctx.enter_context(
                tc.tile_pool(name="pps", bufs=6, space="PSUM"))
            sps = ctx.enter_context(
                tc.tile_pool(name="sps", bufs=2, space="PSUM"))

            def build_kvblocks():
                # block-diagonal kv (with rank-1 bv fix) + den columns
                for ct in range(NCT):
                    for half in range(2):
                        h = 2 * ct + half
                        dst = kvbn[ct][half * HD:(half + 1) * HD,
                                       half * HD:(half + 1) * HD]
                        ks_col = kv_sb[:, h * HE + HD:h * HE + HD + 1]
                        # kv_fixed = bv_h * ksum_h + kv_h  (rank-1 bias fold)
                        nc.vector.scalar_tensor_tensor(
                            dst, bv_rep[0:HD, h * HD:(h + 1) * HD], ks_col,
                            kv_sb[:, h * HE:h * HE + HD], Alu.mult, Alu.add)
                        nc.vector.tensor_copy(
                            kvbn[ct][half * HD:(half + 1) * HD,
                                     128 + half:129 + half],
                            ks_col)

            kf_c = {}
            vp_c = {}

            def r3(t):
                return t[:].rearrange("p (k c) -> p k c", k=NKT)

            def do_k(ci, x8t):
                x3 = r3(x8t)
                w3 = r3(wk_sb)
                kfs = []
                for sub in range(NSUB):
                    ps = pps.tile([P, CG], F32, tag="pps", name="kps")
                    for i in range(NKP):
                        nc.tensor.matmul(
                            ps[:],
                            x3[:, 2 * i:2 * i + 2, sub * P:(sub + 1) * P],
                            w3[:, 2 * i:2 * i + 2, :],
                            start=(i == 0), stop=(i == NKP - 1),
                            perf_mode=DR)
                    # t' = ps + 32*bk (one-ALU-op DVE; descale on ACT)
                    t = tmp.tile([P, CG], BF, tag="t", name="t_kb")
                    nc.vector.tensor_tensor(t[:], ps[:], bk_rep[:], Alu.add)
                    e = tmp.tile([P, CG], BF, tag="t", name="t_e")
                    nc.scalar.activation(e[:], t[:], Act.Exp, scale=INV)
                    r = tmp.tile([P, CG], BF, tag="t", name="t_r")
                    nc.scalar.activation(r[:], t[:], Act.Relu, scale=INV)
                    m = tmp.tile([P, CG], BF, tag="t", name="t_m")
                    nc.vector.tensor_scalar(m[:], e[:], 1.0, None, Alu.min)
                    kf = kfpool.tile([P, CG], BF, tag="kf", name="kf")
                    nc.vector.tensor_tensor(kf[:], m[:], r[:], Alu.add)
                    kfs.append(kf)
                kf_c[ci] = kfs

            def do_v(ci, x8t, xoth):
                vps = []
                if V3:
                    x3, xr3_ = r3(x8t), r3(xoth)
                    wv3, wvr3 = r3(wv_sb), r3(wvr_sb)
                for sub in range(NSUB):
                    ps = pps.tile([P, CG], F32, tag="pps", name="vps")
                    if V3:
                        for i in range(NKP):
                            lhs = x3[:, 2 * i:2 * i + 2,
                                     sub * P:(sub + 1) * P]
                            lhr = xr3_[:, 2 * i:2 * i + 2,
                                       sub * P:(sub + 1) * P]
                            w_ = wv3[:, 2 * i:2 * i + 2, :]
                            wr = wvr3[:, 2 * i:2 * i + 2, :]
                            nc.tensor.matmul(ps[:], lhs, w_,
                                             start=(i == 0), stop=False,
                                             perf_mode=DR)
                            nc.tensor.matmul(ps[:], lhr, w_,
                                             start=False, stop=False,
                                             perf_mode=DR)
                            nc.tensor.matmul(ps[:], lhs, wr,
                                             start=False,
                                             stop=(i == NKP - 1),
                                             perf_mode=DR)
                    else:
                        for kt in range(NKT):
                            c0 = kt * CHUNK + sub * P
                            nc.tensor.matmul(
                                ps[:],
                                xoth[:, c0:c0 + P],
                                wv_sb[:, kt * CG:(kt + 1) * CG],
                                start=(kt == 0), stop=(kt == NKT - 1))
                    vp = vp_tiles[(ci % 2) * NSUB + sub]
                    if V3:
                        nc.scalar.activation(
                            vp[:].rearrange("p (h e) -> p h e",
                                            e=HE)[:, :, :HD],
                            ps[:].rearrange("p (h e) -> p h e", e=HD),
                            Act.Copy, scale=INV)
                    else:
                        # DVE copy: ACT is the tighter engine in the
                        # main loop
                        nc.vector.tensor_copy(
                            vp[:].rearrange("p (h e) -> p h e",
                                            e=HE)[:, :, :HD],
                            ps[:].rearrange("p (h e) -> p h e", e=HD))
                    vps.append(vp)
                vp_c[ci] = vps

            def do_q(ci, x8t):
                x3 = r3(x8t)
                w3 = r3(wq_sb)
                for ct in range(NCT):
                    ps = pps.tile([P, CHUNK], F32, tag="pps", name="qps")
                    for i in range(NKP):
                        nc.tensor.matmul(
                            ps[:],
                            w3[:, 2 * i:2 * i + 2, ct * P:(ct + 1) * P],
                            x3[:, 2 * i:2 * i + 2, :],
                            start=(i == 0), stop=(i == NKP - 1),
                            perf_mode=DR)
                    bcol = bq_sb[:, ct:ct + 1]
                    e = tmp.tile([P, CHUNK], BF, tag="t", name="t_qe")
                    nc.scalar.activation(e[:], ps[:], Act.Exp, bias=bcol,
                                         scale=INV)
                    r = tmp.tile([P, CHUNK], BF, tag="t", name="t_qr")
                    nc.scalar.activation(r[:], ps[:], Act.Relu, bias=bcol,
                                         scale=INV)
                    m = tmp.tile([P, CHUNK], BF, tag="t", name="t_qm")
                    nc.vector.tensor_scalar(m[:], e[:], 1.0, None, Alu.min)
                    q0 = (ci * NCT + ct) * CHUNK
                    nc.vector.tensor_tensor(
                        qft_all[:, q0:q0 + CHUNK], m[:], r[:], Alu.add)

            def do_kv_pair(ci):
                # kv accumulation for chunk pair (ci-1, ci): one psum
                # accumulation group of 8 matmuls per head pair, each in
                # its own bank (regions must NOT share banks: a second
                # region's start=True wipes the first region's data)
                kfs0, vps0 = kf_c.pop(ci - 1), vp_c.pop(ci - 1)
                kfs1, vps1 = kf_c.pop(ci), vp_c.pop(ci)
                for hp in range(NH // 2):
                    kvt = sps.tile([P, 2 * HE], F32, tag="sps", name="kvt")
                    for j, (kfs, vps) in enumerate(((kfs0, vps0),
                                                    (kfs1, vps1))):
                        for sub in range(NSUB):
                            nc.tensor.matmul(
                                kvt[:],
                                kfs[sub][:, hp * P:(hp + 1) * P],
                                vps[sub][:, hp * 2 * HE:(hp + 1) * 2 * HE],
                                start=(j == 0 and sub == 0),
                                stop=(j == 1 and sub == NSUB - 1))
                    # good quadrants: rows 0:64 cols 0:HE (head 2hp),
                    # rows 64:128 cols HE:2HE (head 2hp+1)
                    a0 = (2 * hp) * HE
                    nc.vector.tensor_tensor(
                        kv_sb[:, a0:a0 + HE], kv_sb[:, a0:a0 + HE],
                        kvt[0:HD, 0:HE], Alu.add)
                    a1 = (2 * hp + 1) * HE
                    nc.vector.tensor_tensor(
                        kv_sb[:, a1:a1 + HE], kv_sb[:, a1:a1 + HE],
                        kvt[HD:P, HE:2 * HE], Alu.add)

            # chunk 0/1 interleaved at projection level: k needs only wk,
            # so both chunks' k-projections run while wv/wq still stream in
            if V3:
                xo = {0: xr0, 1: xr1}
            else:
                xo = {0: xb0, 1: xb1}
            x8s = {0: x8t0, 1: x8t1}
            do_k(0, x8s[0])
            do_k(1, x8s[1])
            do_v(0, x8s[0], xo[0])
            do_v(1, x8s[1], xo[1])
            do_q(0, x8s[0])
            do_q(1, x8s[1])
            do_kv_pair(1)
            for ci in range(2, NCHUNK):
                x8t = load_x(ci, x8pool, x8_d, F8, nc.sync)
                if V3:
                    xot = load_x(ci, xrpool, xr_d, F8, nc.gpsimd)
                else:
                    xot = load_x(ci, xbpool, xb_d, BF, nc.gpsimd)
                do_k(ci, x8t)
                do_v(ci, x8t, xot)
                if ci < NCHUNK - 1:
                    do_q(ci, x8t)
                    if ci % 2 == 1:
                        do_kv_pair(ci)
                else:
                    # last chunk: kv first so the kv-block build (DVE)
                    # overlaps the final q matmuls (PE)
                    do_kv_pair(ci)
                    build_kvblocks()
                    do_q(ci, x8t)

            # ---- tail: token-major num (N=128) + den (N=2) sharing the
            # qft stationaries; one [128,32] reciprocal per chunk; the
            # divide-multiply alternates between a direct-psum DVE read
            # and an ACT-copy + bf16 DVE path to balance the engines
            for cj in range(NCHUNK):
                tok0 = cj * CHUNK
                pd = sps.tile([P, 32], F32, tag="sps", name="pd")
                Ts = []
                for sub in range(NSUB):
                    T = pps.tile([P, CG], F32, tag="pps", name="tps")
                    for ct in range(NCT):
                        q0 = (cj * NCT + ct) * CHUNK + sub * P
                        qsl = qft_all[:, q0:q0 + P]
                        nc.tensor.matmul(
                            T[:, ct * P:(ct + 1) * P], qsl,
                            kvbn[ct][:, 0:128], start=True, stop=True)
                        nc.tensor.matmul(
                            pd[:, sub * 8 + ct * 2:sub * 8 + ct * 2 + 2],
                            qsl, kvbn[ct][:, 128:130],
                            start=True, stop=True)
                    Ts.append(T)
                rc = rcpool.tile([P, 32], BF, tag="rc", name="rc")
                with nc.allow_low_precision(
                        reason="bf16 recip: denominators are O(1e3) sums"):
                    nc.vector.reciprocal(rc[:], pd[:])
                for sub in range(NSUB):
                    rcb = rc[:, sub * 8:(sub + 1) * 8].unsqueeze(
                        2).broadcast_to([P, NH, HD])
                    ot = outpool.tile([P, CG], BF, tag="out", name="osb")
                    ot3 = ot[:].rearrange("p (h e) -> p h e", e=HD)
                    if sub % 2 == 0:
                        nc.vector.tensor_tensor(
                            ot3,
                            Ts[sub][:].rearrange("p (h e) -> p h e", e=HD),
                            rcb, Alu.mult)
                    else:
                        pnc = tmp.tile([P, CG], BF, tag="t", name="pnc")
                        nc.scalar.copy(pnc[:], Ts[sub][:])
                        nc.vector.tensor_tensor(
                            ot3,
                            pnc[:].rearrange("p (h e) -> p h e", e=HD),
                            rcb, Alu.mult)
                    nc.sync.dma_start(
                        out_d[tok0 + sub * P:tok0 + (sub + 1) * P, :],
                        ot[:])

    nc.compile()
    return nc


def _get_nc():
    global _CACHED_NC
    if _CACHED_NC is None:
        _CACHED_NC = _build()
    return _CACHED_NC


def _e4(x):
    return np.clip(x, -240.0, 240.0).astype(E4M3)


def _pack_w(w):
    # [D, CG] -> [P, kt-major NKT*CG]: row p, col kt*CG+c = w[kt*P+p, c]
    return np.ascontiguousarray(
        w.reshape(NKT, P, CG).transpose(1, 0, 2).reshape(P, NKT * CG))


def _pack_x(x):
    # [S, D] -> [(chunk, p), kt-major col]:
    # row ci*P+p, col kt*CHUNK+c = X[ci*CHUNK+c, kt*P+p]
    return np.ascontiguousarray(
        x.reshape(NCHUNK, CHUNK, NKT, P)
        .transpose(0, 3, 2, 1).reshape(NCHUNK * P, NKT * CHUNK))


def _make_in_maps(hidden_states, Wq, bq, Wk, bk, Wv, bv):
    hs = np.asarray(hidden_states, np.float32)
    wq = np.asarray(Wq, np.float32)
    wk = np.asarray(Wk, np.float32)
    wv = np.asarray(Wv, np.float32)
    bqf = np.asarray(bq, np.float32)
    # bk is added to the psum BEFORE the 1/32 descale -> pre-scale by 32
    bkf = (WSCALE * np.asarray(bk, np.float32)).astype(BF16)
    bvf = np.asarray(bv, np.float32)
    x8s, xrs, xbs = [], [], []
    for b in range(B):
        xb = hs[b]
        x8 = _e4(xb)
        x8s.append(_pack_x(x8))
        if V3:
            xrs.append(_pack_x(_e4(xb - x8.astype(np.float32))))
        else:
            xbs.append(_pack_x(xb.astype(BF16)))
    in_maps = []
    for c in range(NCORES):
        b, g = divmod(c, 2)
        sl = slice(g * CG, (g + 1) * CG)
        m = {
            "x8": x8s[b],
            "wq": _pack_w(_e4(WSCALE * wq[:, sl])),
            "wk": _pack_w(_e4(WSCALE * wk[:, sl])),
            "bq": np.ascontiguousarray(bqf[sl]),
            "bk": np.ascontiguousarray(bkf[sl]).reshape(1, CG),
            "bv": np.ascontiguousarray(bvf[sl]).reshape(1, CG),
        }
        if V3:
            wvs = WSCALE * wv[:, sl]
            wv8 = _e4(wvs)
            m["xr"] = xrs[b]
            m["wv"] = _pack_w(wv8)
            m["wvr"] = _pack_w(_e4(wvs - wv8.astype(np.float32)))
        else:
            m["xb"] = xbs[b]
            m["wv"] = _pack_w(wv[:, sl].astype(BF16))
        in_maps.append(m)
    return in_maps


def _run(in_maps, **kwargs):
    from concourse.bass_utils import run_bass_kernel_spmd
    nc = _get_nc()
    return run_bass_kernel_spmd(nc, in_maps, core_ids=list(range(NCORES)),
                                **kwargs)


def _assemble(results):
    out = np.empty((B, S, D), np.float32)
    for c in range(NCORES):
        b, g = divmod(c, 2)
        out[b, :, g * CG:(g + 1) * CG] = np.asarray(
            results[c]["out"], dtype=np.float32)
    return out


def kernel(hidden_states, Wq, bq, Wk, bk, Wv, bv):
    in_maps = _make_in_maps(hidden_states, Wq, bq, Wk, bk, Wv, bv)
    res = _run(in_maps)
    return _assemble(res.results)
